# revision 18
# baseline (speedup 1.0000x reference)
"""Trainium2 Bass kernel for nn_FIN_b: windowed-FM tabular net.

Data-parallel over batch: B=2048 rows split across 8 NeuronCores (256 each).
Activations are feature-major ([feature_partition, batch_free]); all matmuls
run in bfloat16.  The windowed FM block
    fm_out[b,c] = 0.5*(sum_e (sum_f x[b,c+f] v[c,f,e])^2 - sum_f x^2 v^2)
is algebraically reduced (the f==f' diagonal cancels) to
    fm_out[b,c] = sum_{d=1..7} sum_f D_d[b, c+f] * G[c, f, f+d],
    D_d = x * shift_d(x),  G[c,f,g] = sum_e v[c,f,e] v[c,g,e].

shift_d(x) is produced on the tensor engine (shifted-identity stationary
operand SA_d sliced from one constant [128, 135] matrix); block-boundary rows
(the d features that wrap into the next 128-feature block) come from a tiny
partition-shifted SBUF DMA plus a GpSimd multiply.  The DVE forms
D_d = x (.) xs_d reading xs straight from PSUM and the banded (block-banded
G weights) matmuls consume D_d immediately, so the PE never idles.  The FM
"linear" term (x_fm @ lin_w) is folded into W1's top half on the host.  The
kernel returns logits; sigmoid (+b2) runs on host.

Other tricks: dummy matmuls warm the PE's HAM clock gate during the initial
weight-DMA wait; ACT tables (relu/lrelu) are preloaded by dummy activations;
the W1 contraction is kt-outer over mt-groups of 4 so W1a/W1b stream from HBM
just ahead of their consumption.
"""

import sys

sys.path.insert(0, "/opt/trn_rl_repo")

import numpy as np
import ml_dtypes

import concourse.bass as bass
import concourse.tile as tile
from concourse import bacc, mybir
from concourse.bass_utils import run_bass_kernel_spmd

NDF, NCF, NCC = 512, 256, 256
EMB, FIELD = 16, 8
B = 2048
NH0 = NDF + 2 * NCC          # 1024
CHANNEL = NH0 - FIELD + 1    # 1017
HID = (NH0 + CHANNEL) // 2   # 1020
NCORES = 8
BC = B // NCORES             # 256 batch rows per core

F32 = mybir.dt.float32
BF16 = mybir.dt.bfloat16

_cache = {}


def _build():
    nc = bacc.Bacc()

    xdT_d = nc.dram_tensor("xdT", [128, 4, BC], BF16, kind="ExternalInput")
    xcT_d = nc.dram_tensor("xcT", [128, 2, BC], BF16, kind="ExternalInput")
    Wd_d = nc.dram_tensor("Wd", [128, 4, NDF], BF16, kind="ExternalInput")
    Wc_d = nc.dram_tensor("Wc", [128, 2, 2 * NCC], BF16, kind="ExternalInput")
    bd_d = nc.dram_tensor("bd", [128, 4], F32, kind="ExternalInput")
    bc_d = nc.dram_tensor("bc", [128, 4], F32, kind="ExternalInput")
    SA_d = nc.dram_tensor("SA", [128, 135], BF16, kind="ExternalInput")
    Gm_d = nc.dram_tensor("Gm", [128, 7, 8, 128], BF16, kind="ExternalInput")
    Gs_d = nc.dram_tensor("Gs", [36, 7, 128], BF16, kind="ExternalInput")
    Gx_d = nc.dram_tensor("Gx", [28, 8, 128], BF16, kind="ExternalInput")
    W1a_d = nc.dram_tensor("W1a", [128, 8, 1024], BF16, kind="ExternalInput")
    W1b_d = nc.dram_tensor("W1b", [128, 8, 1024], BF16, kind="ExternalInput")
    b1_d = nc.dram_tensor("b1", [128, 8], F32, kind="ExternalInput")
    W2_d = nc.dram_tensor("W2", [128, 8, 1], BF16, kind="ExternalInput")
    out_d = nc.dram_tensor("out", [1, BC], F32, kind="ExternalOutput")

    AF = mybir.ActivationFunctionType

    with tile.TileContext(nc) as tc:
        with (
            tc.tile_pool(name="w", bufs=1) as wp,
            tc.tile_pool(name="act", bufs=1) as ap,
            tc.tile_pool(name="acc", bufs=4, space=bass.MemorySpace.PSUM) as pacc,
            tc.tile_pool(name="pxs", bufs=2, space=bass.MemorySpace.PSUM) as pxs,
        ):
            # ---- input DMAs, in need-order (sync HWDGE queue is FIFO) ----
            xdT = wp.tile([128, 4, BC], BF16, tag="xdT")
            nc.sync.dma_start(xdT[:], xdT_d[:])
            Wd = wp.tile([128, 4, NDF], BF16, tag="Wd")
            for kt in range(4):
                nc.sync.dma_start(Wd[:, kt, :], Wd_d[:, kt, :])
            bd = wp.tile([128, 4], F32, tag="bd")
            nc.sync.dma_start(bd[:], bd_d[:])
            xcT = wp.tile([128, 2, BC], BF16, tag="xcT")
            nc.sync.dma_start(xcT[:], xcT_d[:])
            Wc = wp.tile([128, 2, 2 * NCC], BF16, tag="Wc")
            nc.sync.dma_start(Wc[:], Wc_d[:])
            bc = wp.tile([128, 4], F32, tag="bc")
            nc.sync.dma_start(bc[:], bc_d[:])
            SA = wp.tile([128, 135], BF16, tag="SA")
            nc.sync.dma_start(SA[:], SA_d[:])
            Gm = wp.tile([128, 7, 8, 128], BF16, tag="Gm")
            nc.sync.dma_start(Gm[:, 0:2, :, :], Gm_d[:, 0:2, :, :])
            nc.sync.dma_start(Gm[:, 2:4, :, :], Gm_d[:, 2:4, :, :])
            nc.sync.dma_start(Gm[:, 4:7, :, :], Gm_d[:, 4:7, :, :])
            Gs = wp.tile([36, 7, 128], BF16, tag="Gs")
            nc.sync.dma_start(Gs[:], Gs_d[:])
            Gx = wp.tile([28, 8, 128], BF16, tag="Gx")
            nc.sync.dma_start(Gx[:], Gx_d[:])
            W1a = wp.tile([128, 8, 1024], BF16, tag="W1a")
            for kq in range(4):
                nc.sync.dma_start(
                    W1a[:, 2 * kq:2 * kq + 2, :], W1a_d[:, 2 * kq:2 * kq + 2, :])
            W1b = wp.tile([128, 8, 1024], BF16, tag="W1b")
            for kq in range(4):
                nc.sync.dma_start(
                    W1b[:, 2 * kq:2 * kq + 2, :], W1b_d[:, 2 * kq:2 * kq + 2, :])
            b1 = wp.tile([128, 8], F32, tag="b1")
            nc.sync.dma_start(b1[:], b1_d[:])
            W2 = wp.tile([128, 8, 1], BF16, tag="W2")
            nc.sync.dma_start(W2[:], W2_d[:])

            # ---- persistent SBUF activations ----
            xbf = ap.tile([128, 9, BC], BF16, tag="xbf")   # block 8 = zero pad
            nc.vector.memset(xbf[:, 8, :], 0.0)
            D = ap.tile([128, 7, 8, BC], BF16, tag="D")
            XH = ap.tile([28, 8, BC], BF16, tag="XH")      # x[128g+128-d+i]
            XL = ap.tile([28, 8, BC], BF16, tag="XL")      # x[128(g+1)+i]
            ax2 = ap.tile([28, 8, BC], BF16, tag="ax2")    # XH (.) XL
            aux = ap.tile([36, 7, BC], BF16, tag="aux")
            fmb = ap.tile([128, 8, BC], BF16, tag="fmb")
            hb = ap.tile([128, 8, BC], BF16, tag="hb")
            sig = ap.tile([1, BC], F32, tag="sig")
            wsc = ap.tile([128, 2, BC], BF16, tag="wsc")   # warmup scratch
            nc.vector.memset(wsc[:], 0.0)

            # ---- PE warm-up during the initial weight-DMA wait ----
            # ~12 N=512 matmuls on scratch keep the PE busy >3.4us so the HAM
            # clock gate opens (1.2 -> 2.4 GHz) before the real work arrives.
            wps = pxs.tile([128, 4, BC], F32, tag="xs", name="wps")
            for i in range(12):
                nc.tensor.matmul(
                    wps[:, 0:2, :], wsc[:, 0, 0:128], wsc[:],
                    start=True, stop=True,
                )
            # preload ACT tables (relu now, lrelu queued behind the front)
            nc.scalar.activation(
                out=sig[0:1, 0:1], in_=wsc[0:1, 0, 0:1], func=AF.Relu, bias=0.0,
                scale=1.0,
            )

            # ---- front layers: x = relu([Xd,Xc] @ [Wd,Wc] + b) -> xbf ----
            for pair in range(2):            # discrete: mt pairs (0,1), (2,3)
                ps = pacc.tile([128, 2, BC], F32, tag="acc", name=f"fd{pair}")
                for half in range(2):
                    mt = 2 * pair + half
                    for kt in range(4):
                        nc.tensor.matmul(
                            ps[:, half, :],
                            Wd[:, kt, mt * 128:(mt + 1) * 128],
                            xdT[:, kt, :],
                            start=(kt == 0), stop=(kt == 3),
                        )
                    nc.scalar.activation(
                        out=xbf[:, mt, :], in_=ps[:, half, :], func=AF.Relu,
                        bias=bd[:, mt:mt + 1], scale=1.0,
                    )
            for pair in range(2):            # continuous: -> xbf blocks 4..7
                ps = pacc.tile([128, 2, BC], F32, tag="acc", name=f"fc{pair}")
                for half in range(2):
                    mt = 2 * pair + half
                    for kt in range(2):
                        nc.tensor.matmul(
                            ps[:, half, :],
                            Wc[:, kt, mt * 128:(mt + 1) * 128],
                            xcT[:, kt, :],
                            start=(kt == 0), stop=(kt == 1),
                        )
                    nc.scalar.activation(
                        out=xbf[:, 4 + mt, :], in_=ps[:, half, :], func=AF.Relu,
                        bias=bc[:, mt:mt + 1], scale=1.0,
                    )
            # preload the lrelu table while the FM phase runs
            nc.scalar.activation(
                out=sig[0:1, 0:1], in_=wsc[0:1, 0, 0:1], func=AF.Lrelu, bias=0.0,
                scale=1.0, alpha=0.01,
            )

            # ---- FM cross-block pair rows, staged to base-0 partitions ----
            # (scalar/vector HWDGE rings, so these partition-shifted SBUF DMAs
            # don't queue behind the W1 weight stream on sync's ring)
            for d in range(1, 8):
                off = d * (d - 1) // 2
                nc.scalar.dma_start(XH[off:off + d, :, :], xbf[128 - d:128, 0:8, :])
                nc.gpsimd.dma_start(XL[off:off + d, :, :], xbf[0:d, 1:9, :])
            nc.vector.tensor_mul(ax2[:], XH[:], XL[:])

            # ---- FM: shift matmuls -> D_d -> banded G matmuls ----
            # NB: matmul start=True clears has_written bits for the WHOLE
            # PSUM bank (HW-verified), so a bank shared by two accumulation
            # regions must be opened by ONE spanning zero-matmul; every real
            # matmul then accumulates with start=False.
            fm_acc = [
                pacc.tile([128, 2, BC], F32, tag="acc", name=f"fmacc{q}")
                for q in range(4)
            ]
            for q in range(4):
                nc.tensor.matmul(
                    fm_acc[q][:], wsc[:, 0, 0:128], wsc[:],
                    start=True, stop=False, skip_group_check=True,
                )
            for d in range(1, 8):
                for h in range(2):           # half: blocks 4h..4h+3
                    xs = pxs.tile([128, 4, BC], F32, tag="xs", name=f"xs{d}{h}")
                    for q in range(2):       # quarter: blocks (4h+2q, 4h+2q+1)
                        g0 = 4 * h + 2 * q
                        nc.tensor.matmul(
                            xs[:, 2 * q:2 * q + 2, :],
                            SA[:, d:d + 128],
                            xbf[:, g0:g0 + 2, :],
                            start=True, stop=True,
                        )
                    nc.vector.tensor_mul(
                        D[:, d - 1, 4 * h:4 * h + 4, :],
                        xbf[:, 4 * h:4 * h + 4, :],
                        xs[:],
                    )
                # (D wrap rows >= 128-d stay zero; their fm contribution
                # comes from the ax2 / Gx matmul below)
                for Bb in range(8):
                    nc.tensor.matmul(
                        fm_acc[Bb // 2][:, Bb % 2, :],
                        Gm[:, d - 1, Bb, :],
                        D[:, d - 1, Bb, :],
                        start=False, stop=False, skip_group_check=True,
                    )
                if d <= 6:                   # straggler rows for next-block G
                    nc.scalar.dma_start(
                        aux[6 * (d - 1):6 * d, :, :], D[0:6, d - 1, 1:8, :]
                    )
            for Bb in range(7):
                nc.tensor.matmul(
                    fm_acc[Bb // 2][:, Bb % 2, :],
                    Gs[:, Bb, :],
                    aux[:, Bb, :],
                    start=False, stop=False, skip_group_check=True,
                )
            for Bb in range(8):              # cross-block pair contributions
                nc.tensor.matmul(
                    fm_acc[Bb // 2][:, Bb % 2, :],
                    Gx[:, Bb, :],
                    ax2[:, Bb, :],
                    start=False, stop=True, skip_group_check=True,
                )
            for q in range(4):               # evict fm -> bf16 SBUF
                nc.vector.tensor_copy(fmb[:, 2 * q:2 * q + 2, :], fm_acc[q][:])

            # ---- big matmul: h = lrelu(res @ W1 + b1), kt-outer so the
            # W1a/W1b HBM stream is consumed incrementally ----
            for grp in range(2):             # mt groups 0-3 / 4-7
                pss = [
                    pacc.tile([128, 2, BC], F32, tag="acc", name=f"h{grp}{q}")
                    for q in range(2)
                ]
                for q in range(2):
                    nc.tensor.matmul(
                        pss[q][:], wsc[:, 0, 0:128], wsc[:],
                        start=True, stop=False, skip_group_check=True,
                    )
                for kt in range(8):
                    for half in range(4):
                        mt = 4 * grp + half
                        nc.tensor.matmul(
                            pss[half // 2][:, half % 2, :],
                            W1a[:, kt, mt * 128:(mt + 1) * 128],
                            xbf[:, kt, :],
                            start=False, stop=False, skip_group_check=True,
                        )
                for kt in range(8):
                    for half in range(4):
                        mt = 4 * grp + half
                        nc.tensor.matmul(
                            pss[half // 2][:, half % 2, :],
                            W1b[:, kt, mt * 128:(mt + 1) * 128],
                            fmb[:, kt, :],
                            start=False, stop=(kt == 7), skip_group_check=True,
                        )
                for half in range(4):
                    mt = 4 * grp + half
                    nc.scalar.activation(
                        out=hb[:, mt, :], in_=pss[half // 2][:, half % 2, :],
                        func=AF.Lrelu, bias=b1[:, mt:mt + 1], scale=1.0,
                        alpha=0.01,
                    )

            # ---- final: logits = h @ W2 (sigmoid + b2 on host) ----
            psf = pacc.tile([1, BC], F32, tag="acc", name="psf")
            for kt in range(8):
                nc.tensor.matmul(
                    psf[:], W2[:, kt, :], hb[:, kt, :],
                    start=(kt == 0), stop=(kt == 7),
                )
            nc.vector.tensor_copy(sig[:], psf[:])
            nc.sync.dma_start(out_d[:], sig[:])

    nc.finalize()
    return nc


def _prep_shared(inputs):
    """Host-side weight prep shared across cores."""
    Wd = np.asarray(inputs["W_d"], np.float32)
    bd = np.asarray(inputs["b_d"], np.float32)
    Wc = np.asarray(inputs["W_c"], np.float32)
    bc = np.asarray(inputs["b_c"], np.float32)
    v = np.asarray(inputs["v"], np.float32)[0]          # [CHANNEL, FIELD, EMB]
    lin_w = np.asarray(inputs["lin_w"], np.float32)     # [FIELD, 1]
    lin_b = np.asarray(inputs["lin_b"], np.float32)     # [1]
    W1 = np.asarray(inputs["W1"], np.float32)           # [2041, HID]
    b1 = np.asarray(inputs["b1"], np.float32)
    W2 = np.asarray(inputs["W2"], np.float32)           # [HID, 1]

    # banded FM weights: G[c,f,g] = sum_e v[c,f,e] v[c,g,e]
    G = np.einsum("cfe,cge->cfg", v, v)                 # [CHANNEL, 8, 8]
    Gm = np.zeros((128, 7, 8, 128), np.float32)         # [p, d-1, B, m]
    Gs = np.zeros((36, 7, 128), np.float32)             # [6(d-1)+p, B, m]
    m_idx = np.arange(128)
    for d in range(1, 8):
        for Bb in range(8):
            c = 128 * Bb + m_idx                        # [128]
            for f in range(0, 8 - d):
                p = m_idx + f
                ok = (c < CHANNEL) & (p < 128)
                Gm[p[ok], d - 1, Bb, m_idx[ok]] = G[c[ok], f, f + d]
                if Bb < 7:
                    ps_ = p - 128
                    ok2 = (c < CHANNEL) & (ps_ >= 0) & (ps_ < 6)
                    Gs[6 * (d - 1) + ps_[ok2], Bb, m_idx[ok2]] = G[c[ok2], f, f + d]

    # cross-block pair weights: ax2 row off(d)+i is x[j]*x[j+d] for
    # j = 128*Bb + 128-d+i; it feeds fm[c = j-f] (always within block Bb)
    Gx = np.zeros((28, 8, 128), np.float32)
    for d in range(1, 8):
        off = d * (d - 1) // 2
        for i in range(d):
            for f in range(0, 8 - d):
                m = 128 - d + i - f
                for Bb in range(8):
                    c = 128 * Bb + m
                    if c < CHANNEL:
                        Gx[off + i, Bb, m] = G[c, f, f + d]

    # shifted-identity stationary operand: SA[:, d:d+128][k, m] = [k == m+d]
    SA = np.zeros((128, 135), np.float32)
    SA[np.arange(128), np.arange(128)] = 1.0

    # fold the FM linear term (x_fm @ lin_w + lin_b) into W1's top half / b1
    W1a = W1[:NH0].copy()                               # [1024, HID]
    W1b = W1[NH0:]                                      # [CHANNEL, HID]
    for f in range(FIELD):
        W1a[f:f + CHANNEL, :] += lin_w[f, 0] * W1b
    b1e = b1 + lin_b[0] * W1b.sum(0)

    W1a_p = np.zeros((1024, 1024), np.float32)
    W1a_p[:, :HID] = W1a
    W1b_p = np.zeros((1024, 1024), np.float32)
    W1b_p[:CHANNEL, :HID] = W1b
    b1_p = np.zeros(1024, np.float32)
    b1_p[:HID] = b1e
    W2_p = np.zeros(1024, np.float32)
    W2_p[:HID] = W2[:, 0]

    bf = ml_dtypes.bfloat16
    shared = {
        "Wd": np.ascontiguousarray(
            Wd.reshape(4, 128, NDF).transpose(1, 0, 2)).astype(bf),
        "Wc": np.ascontiguousarray(
            Wc.reshape(2, 128, 2 * NCC).transpose(1, 0, 2)).astype(bf),
        "bd": np.ascontiguousarray(bd.reshape(4, 128).T),
        "bc": np.ascontiguousarray(bc.reshape(4, 128).T),
        "SA": SA.astype(bf),
        "Gm": Gm.astype(bf),
        "Gs": Gs.astype(bf),
        "Gx": Gx.astype(bf),
        "W1a": np.ascontiguousarray(
            W1a_p.reshape(8, 128, 1024).transpose(1, 0, 2)).astype(bf),
        "W1b": np.ascontiguousarray(
            W1b_p.reshape(8, 128, 1024).transpose(1, 0, 2)).astype(bf),
        "b1": np.ascontiguousarray(b1_p.reshape(8, 128).T),
        "W2": np.ascontiguousarray(W2_p.reshape(8, 128).T)[:, :, None].astype(bf),
    }
    b2_val = float(np.asarray(inputs["b2"], np.float32)[0])
    return shared, b2_val


def _make_in_maps(inputs):
    dx = np.asarray(inputs["discrete_x"], np.float32)   # [B, NDF]
    cx = np.asarray(inputs["continous_x"], np.float32)  # [B, NCF]
    shared, b2_val = _prep_shared(inputs)
    bf = ml_dtypes.bfloat16
    in_maps = []
    for i in range(NCORES):
        dxi = dx[i * BC:(i + 1) * BC]                   # [BC, NDF]
        cxi = cx[i * BC:(i + 1) * BC]
        m = dict(shared)
        m["xdT"] = np.ascontiguousarray(
            dxi.T.reshape(4, 128, BC).transpose(1, 0, 2)).astype(bf)
        m["xcT"] = np.ascontiguousarray(
            cxi.T.reshape(2, 128, BC).transpose(1, 0, 2)).astype(bf)
        in_maps.append(m)
    return in_maps, b2_val


def kernel(**inputs) -> np.ndarray:
    in_maps, b2_val = _make_in_maps(inputs)

    key = "nc"
    if key not in _cache:
        _cache[key] = _build()
    nc = _cache[key]

    res = run_bass_kernel_spmd(nc, in_maps, core_ids=list(range(NCORES)))
    out = np.empty((B, 1), np.float32)
    for i in range(NCORES):
        out[i * BC:(i + 1) * BC, 0] = res.results[i]["out"][0]
    # device returns logits; sigmoid + b2 here
    out = 1.0 / (1.0 + np.exp(-(out + b2_val)))
    return out
